# revision 1
# baseline (speedup 1.0000x reference)
"""DeSegaMamba (selective-scan vision Mamba) Trainium2 kernel.

kernel(**inputs) takes the FULL inputs of reference.setup_inputs() and
returns the FULL [4, 64, 64, 192] float32 output, running on 8 NeuronCores
(batch-parallel; cores 4-7 mirror batches 0-3).
"""
import numpy as np

"""Patch TileContext._drain_and_barrier: this container's walrus rejects >1
sync-wait per instruction; split the final drain's global-clock waits across
single-wait NOPs on the sync engine."""
import concourse.tile as _tile
from concourse.vector_clock import ScopedClock, VectorClock


def _split_drain_and_barrier(self, tick_clock, wait_clock):
    nc = self.nc
    vclock = tick_clock.global_clock
    n = len(vclock)
    for proc in range(n):
        if vclock[proc] > 0:
            nop = nc.sync.nop(nofuse=True)
            vc = VectorClock([vclock[p] if p == proc else 0 for p in range(n)])
            wait_clock.add_sem_waits(nop.ins, ScopedClock({None: vc}))
    nc.sync.drain()
    nc.all_engine_barrier()
    popped = nc._tile_sem_poison_stack.pop()
    assert popped is self._sem_poison
    nc.clear_and_free_semaphores(list(self.sems.allocated().values()))
    nc.all_engine_barrier()


def apply():
    _tile.TileContext._drain_and_barrier = _split_drain_and_barrier


def legalize_waits(nc, max_waits=1):
    """Walrus (this container's version) rejects >1 sync-wait per instruction.
    Split extras onto preceding same-engine NoOps."""
    from concourse import mybir
    n_split = 0
    for f in nc.m.functions:
        for blk in f.blocks:
            insts = blk.instructions
            out = []
            changed = False
            for inst in insts:
                si = inst.sync_info
                if si is not None and si.on_wait and len(si.on_wait) > max_waits:
                    waits = list(si.on_wait)
                    extra, keep = waits[:-max_waits], waits[-max_waits:]
                    for w_i, w in enumerate(extra):
                        nop = mybir.InstNoOp(
                            name=f"{inst.name}_wsplit{w_i}", ins=[], outs=[])
                        nop.engine = inst.engine
                        nop.sync_info = mybir.SyncInfo(on_wait=[w],
                                                       on_update=[])
                        nc.register_instruction(nop, overwrite=True)
                        out.append(nop)
                    si.on_wait = keep
                    inst.sync_info = si
                    changed = True
                    n_split += 1
                out.append(inst)
            if changed:
                blk.instructions = out
    return n_split



from contextlib import ExitStack

import numpy as np

import concourse.bass as bass
import concourse.tile as tile
from concourse import mybir

F32 = mybir.dt.float32
BF16 = mybir.dt.bfloat16
AF = mybir.ActivationFunctionType
OP = mybir.AluOpType

L = 4096
C = 192          # d_model
DI = 384         # d_inner
NS = 16          # d_state
RK = 12          # dt_rank
WIMG = 64        # image width (and height)
TC = 512         # time chunk
NCH = L // TC    # 8
NJ = 3           # d_inner row blocks of 128
PAD = 66         # left/right pad of xiT


def build_kernel(cfg=None):
    cfg = cfg or {}
    h_dt = BF16 if cfg.get("h_bf16", True) else F32
    x_dt = BF16 if cfg.get("x_bf16", True) else F32
    bc_dt = BF16 if cfg.get("bc_bf16", True) else F32   # B/C rows dtype

    nc = bass.Bass()

    # ---- DRAM I/O ----
    xb = nc.dram_tensor("xb", [L, C], F32, kind="ExternalInput")
    pb = nc.dram_tensor("pb", [L, C], F32, kind="ExternalInput")
    W_in = nc.dram_tensor("W_in", [C, 2 * DI], F32, kind="ExternalInput")
    cw9 = nc.dram_tensor("cw9", [DI, 9], F32, kind="ExternalInput")
    wcols = nc.dram_tensor("wcols", [DI, 5], F32, kind="ExternalInput")
    Wx = nc.dram_tensor("Wx", [DI, RK + 2 * NS], F32, kind="ExternalInput")
    Wdt = nc.dram_tensor("Wdt", [RK, DI], F32, kind="ExternalInput")
    Alog = nc.dram_tensor("Alog", [DI, NS], F32, kind="ExternalInput")
    Wp = nc.dram_tensor("Wp", [C, NS], F32, kind="ExternalInput")
    Wout = nc.dram_tensor("Wout", [DI, C], F32, kind="ExternalInput")
    eye = nc.dram_tensor("eye128", [128, 128], F32, kind="ExternalInput")
    cwdiag = nc.dram_tensor("cwdiag", [27, 128, 128], BF16,
                            kind="ExternalInput")
    yb = nc.dram_tensor("yb", [L, C], F32, kind="ExternalOutput")

    # internal DRAM bounce for B/C rows and LN stats (partition-broadcast src)
    z_d = nc.dram_tensor("z_d", [DI, L], BF16)
    bs_d = nc.dram_tensor("bs_d", [NS, L], bc_dt)
    cs_d = nc.dram_tensor("cs_d", [NS, L], bc_dt)
    mu_d = nc.dram_tensor("mu_d", [1, L], F32)
    rs_d = nc.dram_tensor("rs_d", [1, L], F32)

    if cfg.get("trivial"):
        with tile.TileContext(nc) as tc, ExitStack() as ctx:
            pool = ctx.enter_context(tc.tile_pool(name="triv", bufs=4))
            for i in range(L // 128):
                t = pool.tile([128, C], F32, tag="t", name=f"t{i}")
                nc.sync.dma_start(out=t, in_=xb[i * 128:(i + 1) * 128, :])
                nc.sync.dma_start(out=yb[i * 128:(i + 1) * 128, :], in_=t)
        return nc

    with tile.TileContext(nc) as tc, ExitStack() as ctx:
        wpool = ctx.enter_context(tc.tile_pool(name="weights", bufs=1))
        upool = ctx.enter_context(tc.tile_pool(name="upool", bufs=1))
        dpool = ctx.enter_context(tc.tile_pool(name="dpool", bufs=1))
        ld = ctx.enter_context(tc.tile_pool(name="ld", bufs=2))
        sm = ctx.enter_context(tc.tile_pool(name="sm", bufs=2))
        statp = ctx.enter_context(tc.tile_pool(name="statp", bufs=4))
        esbp = ctx.enter_context(tc.tile_pool(name="esbp", bufs=2))
        ps_tr = ctx.enter_context(tc.tile_pool(name="ps_tr", bufs=2, space="PSUM"))
        ps_mm = ctx.enter_context(tc.tile_pool(name="ps_mm", bufs=3, space="PSUM"))
        ps_y = ctx.enter_context(tc.tile_pool(name="ps_y", bufs=1, space="PSUM"))

        # ---------------- weights to SBUF ----------------
        eye_t = wpool.tile([128, 128], F32, tag="eye")
        nc.gpsimd.dma_start(out=eye_t, in_=eye[:, :])
        eye_b = wpool.tile([128, 128], BF16, tag="eyeb")
        nc.vector.tensor_copy(out=eye_b, in_=eye_t)
        ones_t = wpool.tile([128, 1], F32, tag="ones")
        nc.vector.memset(ones_t, 1.0)
        zcol = wpool.tile([128, 1], F32, tag="zcol")
        nc.vector.memset(zcol, 0.0)
        epscol = wpool.tile([1, 1], F32, tag="eps")
        nc.vector.memset(epscol, 1e-5)

        cw_t, cwn_t, An_t, Wx_t, Wout_t = [], [], [], [], []
        # packed loads: [DI, X] -> [128, NJ, X] in one DMA each
        def packed(dram, X, tg, dt=F32):
            t = wpool.tile([128, NJ, X], dt, tag=tg, name=tg)
            ap = dram[0:1, 0:1]
            nc.gpsimd.dma_start(out=t, in_=bass.AP(
                tensor=ap.tensor, offset=0,
                ap=[[X, 128], [128 * X, NJ], [1, X]]))
            return t

        cw_all = packed(cw9, 9, "cw_all")
        wc_all = packed(wcols, 5, "wc_all")
        al_all = packed(Alog, NS, "al_all")
        wx_allf = packed(Wx, RK + 2 * NS, "wx_allf")
        wo_allf = packed(Wout, C, "wo_allf")
        wx_all = wpool.tile([128, NJ, RK + 2 * NS], BF16, tag="wx_all")
        nc.vector.tensor_copy(out=wx_all, in_=wx_allf)
        wo_all = wpool.tile([128, NJ, C], BF16, tag="wo_all")
        nc.vector.tensor_copy(out=wo_all, in_=wo_allf)
        cwd_all = wpool.tile([128, 27, 128], BF16, tag="cwd_all")
        ap = cwdiag[0:1, 0:1, 0:1]
        nc.gpsimd.dma_start(out=cwd_all, in_=bass.AP(
            tensor=ap.tensor, offset=0,
            ap=[[128, 128], [16384, 27], [1, 128]]))
        cwd_t = [[cwd_all[:, j * 9 + tap, :] for tap in range(9)]
                 for j in range(NJ)]
        an_all = wpool.tile([128, NJ, NS], F32, tag="an_all")
        nc.scalar.activation(out=an_all, in_=al_all, func=AF.Exp)
        nc.vector.tensor_scalar_mul(out=an_all, in0=an_all, scalar1=-1.0)
        cwn_all = wpool.tile([128, NJ, 9], F32, tag="cwn_all")
        nc.vector.tensor_scalar_mul(out=cwn_all, in0=cw_all, scalar1=-1.0)
        cb_t, bdt_t, D_t, g_t, b_t = [], [], [], [], []
        for j in range(NJ):
            cw_t.append(cw_all[:, j, :])
            cwn_t.append(cwn_all[:, j, :])
            cb_t.append(wc_all[:, j, 0:1])
            bdt_t.append(wc_all[:, j, 1:2])
            D_t.append(wc_all[:, j, 2:3])
            g_t.append(wc_all[:, j, 3:4])
            b_t.append(wc_all[:, j, 4:5])
            An_t.append(an_all[:, j, :])
            Wx_t.append(wx_all[:, j, :])
            Wout_t.append(wo_all[:, j, :])
        Wdt_t = wpool.tile([RK, DI], F32, tag="wdt")
        nc.gpsimd.dma_start(out=Wdt_t, in_=Wdt[:, :])
        Wp_a = wpool.tile([128, NS], F32, tag="wp_a")
        nc.gpsimd.dma_start(out=Wp_a, in_=Wp[0:128, :])
        Wp_b = wpool.tile([64, NS], F32, tag="wp_b")
        nc.gpsimd.dma_start(out=Wp_b, in_=Wp[128:192, :])

        uT16 = [upool.tile([128, L], BF16, tag=f"uT16{j}", name=f"uT16{j}")
                for j in range(NJ)]
        deltaT = [[dpool.tile([128, TC], BF16, tag=f"deltaT{j}_{k}",
                              name=f"deltaT{j}_{k}") for k in range(NCH)]
                  for j in range(NJ)]

        PW = PAD + L + PAD

        if True:

            with tc.tile_pool(name="xipool", bufs=1) as xipool:
                xiT = [xipool.tile([128, PW], BF16, tag=f"xiT{j}",
                                   name=f"xiT{j}") for j in range(NJ)]

                with tc.tile_pool(name="xtpool", bufs=1) as xtpool:
                    xTa = [xtpool.tile([128, TC], BF16, tag=f"xTa{k}",
                                       name=f"xTa{k}") for k in range(NCH)]
                    xTb = [xtpool.tile([64, TC], BF16, tag=f"xTb{k}",
                                       name=f"xTb{k}") for k in range(NCH)]
                    Wi_af = xtpool.tile([128, 2 * DI], F32, tag="Wi_af")
                    Wi_bf = xtpool.tile([64, 2 * DI], F32, tag="Wi_bf")
                    nc.gpsimd.dma_start(out=Wi_af, in_=W_in[0:128, :])
                    nc.gpsimd.dma_start(out=Wi_bf, in_=W_in[128:192, :])
                    Wi_a = xtpool.tile([128, 2 * DI], BF16, tag="Wi_a")
                    Wi_b = xtpool.tile([64, 2 * DI], BF16, tag="Wi_b")
                    nc.vector.tensor_copy(out=Wi_a, in_=Wi_af)
                    nc.vector.tensor_copy(out=Wi_b, in_=Wi_bf)
                    # ---- phase 1: transpose x ----
                    for i in range(L // 128):
                        kk, cc = i // 4, (i % 4) * 128
                        tin = ld.tile([128, C], F32, tag="tin")
                        nc.sync.dma_start(out=tin,
                                          in_=xb[i * 128:(i + 1) * 128, :])
                        pt1 = ps_tr.tile([128, 128], F32, tag="pt")
                        nc.tensor.transpose(out=pt1, in_=tin[:, 0:128],
                                            identity=eye_t)
                        nc.scalar.copy(out=xTa[kk][:, cc:cc + 128], in_=pt1)
                        pt2 = ps_tr.tile([64, 128], F32, tag="pt")
                        nc.tensor.transpose(out=pt2, in_=tin[:, 128:192],
                                            identity=eye_t)
                        nc.scalar.copy(out=xTb[kk][:, cc:cc + 128], in_=pt2)

                    # ---- phase 2: xz projection ----
                    for j in range(NJ):
                        nc.vector.memset(xiT[j][:, 0:PAD], 0.0)
                        nc.vector.memset(xiT[j][:, PAD + L:PW], 0.0)
                    for j in range(NJ):
                        c0 = j * 128
                        for k in range(NCH):
                            t0 = k * TC
                            pxi = ps_mm.tile([128, TC], F32, tag="pmm")
                            nc.tensor.matmul(pxi,
                                             Wi_a[:, c0:c0 + 128],
                                             xTa[k][:, :],
                                             start=True, stop=False)
                            nc.tensor.matmul(pxi,
                                             Wi_b[:, c0:c0 + 128],
                                             xTb[k][:, :],
                                             start=False, stop=True)
                            nc.scalar.copy(
                                out=xiT[j][:, PAD + t0:PAD + t0 + TC],
                                in_=pxi)
                            pz = ps_mm.tile([128, TC], F32, tag="pmm")
                            nc.tensor.matmul(
                                pz, Wi_a[:, DI + c0:DI + c0 + 128],
                                xTa[k][:, :],
                                start=True, stop=False)
                            nc.tensor.matmul(
                                pz, Wi_b[:, DI + c0:DI + c0 + 128],
                                xTb[k][:, :],
                                start=False, stop=True)
                            zs = sm.tile([128, TC], BF16, tag="zs",
                                          name=f"zs{j}_{k}")
                            nc.scalar.activation(out=zs, in_=pz, func=AF.Silu)
                            nc.sync.dma_start(
                                out=z_d[j * 128:(j + 1) * 128, t0:t0 + TC],
                                in_=zs)

                # ---- phase 3: depthwise conv via diagonal matmuls on PE ----
                for j in range(NJ):
                    for k in range(NCH):
                        t0 = k * TC
                        pc = ps_mm.tile([128, TC], F32, tag="pmm",
                                        name=f"pconv{j}_{k}")
                        for ky in range(3):
                            for kx in range(3):
                                dy, dx = ky - 1, kx - 1
                                tap = ky * 3 + kx
                                off = PAD + t0 + dy * WIMG + dx
                                nc.tensor.matmul(
                                    pc, cwd_t[j][tap],
                                    xiT[j][:, off:off + TC],
                                    start=(tap == 0), stop=(tap == 8))
                        acc = ld.tile([128, TC], F32, tag="cacc",
                                      name=f"cacc{j}_{k}")
                        nc.scalar.copy(out=acc, in_=pc)
                        # wrap fixes within this chunk (8 image rows):
                        # dx=+1 contaminates w=63, dx=-1 contaminates w=0
                        a3 = acc.rearrange("p (h w) -> p h w", w=WIMG)
                        xi3 = xiT[j]
                        r0 = t0 // WIMG  # first image row of this chunk
                        eng = nc.vector
                        for ky in range(3):
                            dy = ky - 1

                            def stv(start):
                                base = xi3[:, start:start + 1]
                                return bass.AP(
                                    tensor=base.tensor, offset=base.offset,
                                    ap=[list(base.ap[0]),
                                        [WIMG, TC // WIMG]])

                            tap = ky * 3 + 2   # dx=+1 read xi[h+dy+1, 0]
                            eng.scalar_tensor_tensor(
                                out=a3[:, :, WIMG - 1],
                                in0=stv(PAD + (r0 + dy + 1) * WIMG),
                                scalar=cwn_t[j][:, tap:tap + 1],
                                in1=a3[:, :, WIMG - 1],
                                op0=OP.mult, op1=OP.add)
                            tap = ky * 3       # dx=-1 read xi[h+dy-1, 63]
                            eng.scalar_tensor_tensor(
                                out=a3[:, :, 0],
                                in0=stv(PAD + (r0 + dy) * WIMG - 1),
                                scalar=cwn_t[j][:, tap:tap + 1],
                                in1=a3[:, :, 0],
                                op0=OP.mult, op1=OP.add)
                        nc.scalar.activation(out=uT16[j][:, t0:t0 + TC],
                                             in_=acc, func=AF.Silu,
                                             bias=cb_t[j])

            # ---- phase 3.5 + 4: prompt transpose; x_dbl, delta, B, C ----
            with tc.tile_pool(name="ptpool", bufs=1) as ptpool:
                pT_a = ptpool.tile([128, L], F32, tag="pT_a")
                pT_b = ptpool.tile([64, L], F32, tag="pT_b")
                dtrT = ptpool.tile([RK, L], F32, tag="dtrT")
                for i in range(L // 128):
                    tin = ld.tile([128, C], F32, tag="tin")
                    nc.sync.dma_start(out=tin, in_=pb[i * 128:(i + 1) * 128, :])
                    pt1 = ps_tr.tile([128, 128], F32, tag="pt")
                    nc.tensor.transpose(out=pt1, in_=tin[:, 0:128],
                                        identity=eye_t)
                    nc.scalar.copy(out=pT_a[:, i * 128:(i + 1) * 128], in_=pt1)
                    pt2 = ps_tr.tile([64, 128], F32, tag="pt")
                    nc.tensor.transpose(out=pt2, in_=tin[:, 128:192],
                                        identity=eye_t)
                    nc.scalar.copy(out=pT_b[:, i * 128:(i + 1) * 128], in_=pt2)

                for k in range(NCH):
                    t0 = k * TC
                    p_dtr = ps_mm.tile([RK, TC], F32, tag="pmm")
                    p_B = ps_mm.tile([NS, TC], F32, tag="pmm")
                    p_C = ps_mm.tile([NS, TC], F32, tag="pmm")
                    for j in range(NJ):
                        nc.tensor.matmul(p_dtr, Wx_t[j][:, 0:RK],
                                         uT16[j][:, t0:t0 + TC],
                                         start=(j == 0), stop=(j == NJ - 1))
                    for j in range(NJ):
                        nc.tensor.matmul(p_B, Wx_t[j][:, RK:RK + NS],
                                         uT16[j][:, t0:t0 + TC],
                                         start=(j == 0), stop=(j == NJ - 1))
                    for j in range(NJ):
                        nc.tensor.matmul(
                            p_C, Wx_t[j][:, RK + NS:RK + 2 * NS],
                            uT16[j][:, t0:t0 + TC],
                            start=(j == 0), stop=False)
                    nc.tensor.matmul(p_C, Wp_a[:, :],
                                     pT_a[:, t0:t0 + TC],
                                     start=False, stop=False)
                    nc.tensor.matmul(p_C, Wp_b[:, :],
                                     pT_b[:, t0:t0 + TC],
                                     start=False, stop=True)
                    nc.scalar.copy(out=dtrT[:, t0:t0 + TC], in_=p_dtr)
                    bt = sm.tile([NS, TC], bc_dt, tag="bs16")
                    nc.scalar.copy(out=bt, in_=p_B)
                    nc.sync.dma_start(out=bs_d[:, t0:t0 + TC], in_=bt)
                    ct = sm.tile([NS, TC], bc_dt, tag="bs16")
                    nc.scalar.copy(out=ct, in_=p_C)
                    nc.sync.dma_start(out=cs_d[:, t0:t0 + TC], in_=ct)

                for j in range(NJ):
                    for k in range(NCH):
                        t0 = k * TC
                        pd = ps_mm.tile([128, TC], F32, tag="pmm")
                        nc.tensor.matmul(
                            pd, Wdt_t[:, j * 128:(j + 1) * 128],
                            dtrT[:, t0:t0 + TC],
                            start=True, stop=True)
                        # softplus(x) = ln(1 + exp(x)) via Exp then Ln
                        esb = esbp.tile([128, TC], F32, tag="esb",
                                      name=f"esb{j}_{k}")
                        nc.scalar.activation(out=esb, in_=pd, func=AF.Exp,
                                             bias=bdt_t[j])
                        nc.scalar.activation(out=deltaT[j][k],
                                             in_=esb, func=AF.Ln, bias=1.0)


        # ---------------- phase 6+7: scan + LN + gate + out ----------------
        hpool = ctx.enter_context(tc.tile_pool(name="hpool", bufs=1))
        sc = ctx.enter_context(tc.tile_pool(name="scan", bufs=6))
        scx = ctx.enter_context(tc.tile_pool(name="scanx", bufs=4))
        scW = ctx.enter_context(tc.tile_pool(name="scanw", bufs=2))
        bc = ctx.enter_context(tc.tile_pool(name="bc", bufs=2))
        bcmu = ctx.enter_context(tc.tile_pool(name="bcmu", bufs=1))
        yc_pool = ctx.enter_context(tc.tile_pool(name="ycp", bufs=3))
        ysq_pool = ctx.enter_context(tc.tile_pool(name="ysqp", bufs=2))
        zl_pool = ctx.enter_context(tc.tile_pool(name="zl", bufs=2))
        out_pool = ctx.enter_context(tc.tile_pool(name="outp", bufs=2))
        hst = [[hpool.tile([128, TC], h_dt, tag=f"h{j}_{n}",
                           name=f"h{j}_{n}") for n in range(NS)]
               for j in range(NJ)]
        for k in range(NCH):
            t0 = k * TC
            py = [ps_y.tile([128, TC], F32, tag=f"py{j}", name=f"py{j}_{k}")
                  for j in range(NJ)]
            wch = []
            for j in range(NJ):
                w_t = scW.tile([128, TC], BF16, tag=f"w{j}", name=f"w{j}_{k}")
                nc.gpsimd.tensor_tensor(out=w_t, in0=deltaT[j][k],
                                        in1=uT16[j][:, t0:t0 + TC],
                                        op=OP.mult)
                wch.append(w_t)
            for n in range(NS):
                brow = bc.tile([128, TC], bc_dt, tag="brow")
                bsrc = bs_d[n:n + 1, t0:t0 + TC]
                nc.gpsimd.dma_start(out=brow, in_=bass.AP(
                    tensor=bsrc.tensor, offset=bsrc.offset,
                    ap=[[0, 128]] + list(bsrc.ap[1:])))
                crow = bc.tile([128, TC], bc_dt, tag="crow")
                csrc = cs_d[n:n + 1, t0:t0 + TC]
                nc.gpsimd.dma_start(out=crow, in_=bass.AP(
                    tensor=csrc.tensor, offset=csrc.offset,
                    ap=[[0, 128]] + list(csrc.ap[1:])))
                for j in range(NJ):
                    a_t = sc.tile([128, TC], F32, tag="a")
                    nc.scalar.activation(out=a_t, in_=deltaT[j][k],
                                         func=AF.Exp,
                                         scale=An_t[j][:, n:n + 1])
                    x_t = scx.tile([128, TC], x_dt, tag="x")
                    xb_eng = nc.gpsimd if j == 2 else nc.vector
                    xb_eng.tensor_tensor(out=x_t, in0=wch[j], in1=brow,
                                         op=OP.mult)
                    h = hst[j][n]
                    init = zcol[:, 0:1] if k == 0 else h[:, TC - 1:TC]
                    nc.vector.tensor_tensor_scan(
                        out=h, data0=a_t, data1=x_t, initial=init,
                        op0=OP.mult, op1=OP.add)
                    ym = scx.tile([128, TC], h_dt, tag="ym")
                    ym_eng = nc.gpsimd if (j >= 1 and cfg.get(
                        "ymul_pool", True)) else nc.vector
                    ym_eng.tensor_tensor(out=ym, in0=h, in1=crow, op=OP.mult)
                    nc.tensor.matmul(py[j], eye_b if h_dt == BF16 else eye_t,
                                     ym, start=(n == 0), stop=(n == NS - 1))

            # y chunk = py + u*D ; LayerNorm stats via ones-matmul
            yc = []
            p_s = ps_mm.tile([1, TC], F32, tag="pmm")
            p_q = ps_mm.tile([1, TC], F32, tag="pmm")
            for j in range(NJ):
                t = yc_pool.tile([128, TC], F32, tag="yc", name=f"yc{j}_{k}")
                nc.vector.scalar_tensor_tensor(
                    out=t, in0=uT16[j][:, t0:t0 + TC],
                    scalar=D_t[j], in1=py[j],
                    op0=OP.mult, op1=OP.add)
                yc.append(t)
            ysq = []
            for j in range(NJ):
                q = ysq_pool.tile([128, TC], F32, tag="ysq", name=f"ysq{j}_{k}")
                nc.gpsimd.tensor_tensor(out=q, in0=yc[j], in1=yc[j],
                                        op=OP.mult)
                ysq.append(q)
            for j in range(NJ):
                nc.tensor.matmul(p_s, ones_t[:, 0:1], yc[j],
                                 start=(j == 0), stop=(j == NJ - 1))
            for j in range(NJ):
                nc.tensor.matmul(p_q, ones_t[:, 0:1], ysq[j],
                                 start=(j == 0), stop=(j == NJ - 1))
            mu = statp.tile([1, TC], F32, tag="stat", name=f"mu{k}")
            nc.scalar.mul(out=mu, in_=p_s, mul=1.0 / DI)
            msq = statp.tile([1, TC], F32, tag="stat", name=f"msq{k}")
            nc.scalar.mul(out=msq, in_=p_q, mul=1.0 / DI)
            var = statp.tile([1, TC], F32, tag="stat", name=f"var{k}")
            nc.vector.tensor_tensor(out=var, in0=mu, in1=mu, op=OP.mult)
            nc.vector.tensor_tensor(out=var, in0=msq, in1=var, op=OP.subtract)
            lnv = statp.tile([1, TC], F32, tag="stat", name=f"lnv{k}")
            nc.scalar.activation(out=lnv, in_=var, func=AF.Ln,
                                 bias=epscol[0:1, 0:1])
            rstd = statp.tile([1, TC], F32, tag="stat", name=f"rstd{k}")
            nc.scalar.activation(out=rstd, in_=lnv, func=AF.Exp, scale=-0.5)
            nc.sync.dma_start(out=mu_d[:, t0:t0 + TC], in_=mu)
            nc.sync.dma_start(out=rs_d[:, t0:t0 + TC], in_=rstd)
            mu_b = bcmu.tile([128, TC], F32, tag="mu_b")
            rs_b = bcmu.tile([128, TC], F32, tag="rs_b")
            src = mu_d[0:1, t0:t0 + TC]
            nc.gpsimd.dma_start(out=mu_b, in_=bass.AP(
                tensor=src.tensor, offset=src.offset,
                ap=[[0, 128]] + list(src.ap[1:])))
            src = rs_d[0:1, t0:t0 + TC]
            nc.gpsimd.dma_start(out=rs_b, in_=bass.AP(
                tensor=src.tensor, offset=src.offset,
                ap=[[0, 128]] + list(src.ap[1:])))

            po_a = ps_mm.tile([128, TC], F32, tag="pmm")
            po_b = ps_mm.tile([64, TC], F32, tag="pmm")
            for j in range(NJ):
                g1 = out_pool.tile([128, TC], F32, tag="g1", name=f"g1{j}_{k}")
                g16 = out_pool.tile([128, TC], BF16, tag="g16",
                                    name=f"g16{j}_{k}")
                ln_eng = nc.gpsimd if j >= 1 else nc.vector
                ln_eng.tensor_tensor(out=g1, in0=yc[j], in1=mu_b,
                                     op=OP.subtract)
                ln_eng.tensor_tensor(out=g1, in0=g1, in1=rs_b, op=OP.mult)
                ln_eng.tensor_scalar(out=g1, in0=g1,
                                     scalar1=g_t[j],
                                     scalar2=b_t[j],
                                     op0=OP.mult, op1=OP.add)
                zl = zl_pool.tile([128, TC], BF16, tag="zl",
                                  name=f"zl{j}_{k}")
                nc.sync.dma_start(out=zl,
                                  in_=z_d[j * 128:(j + 1) * 128, t0:t0 + TC])
                ln_eng.tensor_tensor(out=g16, in0=g1, in1=zl, op=OP.mult)
                nc.tensor.matmul(po_a, Wout_t[j][:, 0:128], g16,
                                 start=(j == 0), stop=(j == NJ - 1))
                nc.tensor.matmul(po_b, Wout_t[j][:, 128:192], g16,
                                 start=(j == 0), stop=(j == NJ - 1))
            so_a = out_pool.tile([128, TC], F32, tag="so", name=f"so_a{k}")
            nc.scalar.copy(out=so_a, in_=po_a)
            so_b = out_pool.tile([64, TC], F32, tag="so", name=f"so_b{k}")
            nc.scalar.copy(out=so_b, in_=po_b)
            for ci in range(TC // 128):
                ot = out_pool.tile([128, C], F32, tag="ot",
                                   name=f"ot{k}_{ci}")
                pa = ps_tr.tile([128, 128], F32, tag="pt")
                nc.tensor.transpose(out=pa,
                                    in_=so_a[:, ci * 128:(ci + 1) * 128],
                                    identity=eye_t)
                nc.scalar.copy(out=ot[:, 0:128], in_=pa)
                pb2 = ps_tr.tile([128, 64], F32, tag="pt")
                nc.tensor.transpose(out=pb2,
                                    in_=so_b[:, ci * 128:(ci + 1) * 128],
                                    identity=eye_t[0:64, 0:64])
                nc.scalar.copy(out=ot[:, 128:192], in_=pb2)
                nc.sync.dma_start(
                    out=yb[t0 + ci * 128:t0 + (ci + 1) * 128, :], in_=ot)

    return nc


def make_in_map(inputs, b):
    """Per-core input dict for batch b from full reference inputs."""
    f32 = np.float32
    x = np.asarray(inputs["x"], f32)
    prompt = np.asarray(inputs["prompt"], f32)
    wcols = np.stack([
        np.asarray(inputs["conv_b"], f32).reshape(DI),
        np.asarray(inputs["b_dt"], f32).reshape(DI),
        np.asarray(inputs["D"], f32).reshape(DI),
        np.asarray(inputs["ln_g"], f32).reshape(DI),
        np.asarray(inputs["ln_b"], f32).reshape(DI),
    ], axis=1)  # [DI, 5]
    return {
        "xb": np.ascontiguousarray(x[b].reshape(L, C)),
        "pb": np.ascontiguousarray(prompt[b].reshape(L, C)),
        "W_in": np.asarray(inputs["W_in"], f32),
        "cw9": np.ascontiguousarray(np.asarray(inputs["conv_w"], f32)
                                    .reshape(DI, 9)),
        "wcols": np.ascontiguousarray(wcols),
        "Wx": np.asarray(inputs["Wx"], f32),
        "Wdt": np.asarray(inputs["Wdt"], f32),
        "Alog": np.asarray(inputs["A_log"], f32),
        "Wp": np.asarray(inputs["Wp"], f32),
        "Wout": np.asarray(inputs["Wout"], f32),
        "eye128": np.eye(128, dtype=f32),
        "cwdiag": _cwdiag(np.asarray(inputs["conv_w"], f32).reshape(DI, 9)),
    }


def _cwdiag(cw9):
    import ml_dtypes
    out = np.zeros((27, 128, 128), dtype=ml_dtypes.bfloat16)
    for j in range(NJ):
        for tap in range(9):
            np.fill_diagonal(out[j * 9 + tap],
                             cw9[j * 128:(j + 1) * 128, tap]
                             .astype(ml_dtypes.bfloat16))
    return out



_RUNNER_CACHE = {}


def _get_runner():
    if "r" in _RUNNER_CACHE:
        return _RUNNER_CACHE["r"]
    apply()  # tile drain patch
    import jax
    from jax.experimental.shard_map import shard_map
    from jax.sharding import Mesh, PartitionSpec
    from concourse import bass2jax as b2j

    nc = build_kernel()
    legalize_waits(nc)
    b2j.install_neuronx_cc_hook()
    partition_name = (nc.partition_id_tensor.name
                      if nc.partition_id_tensor else None)
    in_names, out_names, out_avals, zero_outs = [], [], [], []
    for alloc in nc.m.functions[0].allocations:
        if not isinstance(alloc, mybir.MemoryLocationSet):
            continue
        name = alloc.memorylocations[0].name
        if alloc.kind == "ExternalInput":
            if name != partition_name:
                in_names.append(name)
        elif alloc.kind == "ExternalOutput":
            out_names.append(name)
            shape = tuple(alloc.tensor_shape)
            dtype = mybir.dt.np(alloc.dtype)
            out_avals.append(jax.core.ShapedArray(shape, dtype))
            zero_outs.append(np.zeros(shape, dtype))
    n_params = len(in_names)
    all_in_names = list(in_names) + list(out_names)
    if partition_name is not None:
        all_in_names.append(partition_name)

    def _body(*args):
        operands = list(args)
        if partition_name is not None:
            operands.append(b2j.partition_id_tensor())
        outs = b2j._bass_exec_p.bind(
            *operands,
            out_avals=tuple(out_avals),
            in_names=tuple(all_in_names),
            out_names=tuple(out_names),
            lowering_input_output_aliases=(),
            sim_require_finite=True,
            sim_require_nnan=True,
            nc=nc,
        )
        return tuple(outs)

    n_cores = 8
    devices = jax.devices()[:n_cores]
    mesh = Mesh(np.asarray(devices), ("core",))
    in_specs = (PartitionSpec("core"),) * (n_params + len(out_names))
    out_specs = (PartitionSpec("core"),) * len(out_names)
    fn = jax.jit(
        shard_map(_body, mesh=mesh, in_specs=in_specs,
                  out_specs=out_specs, check_rep=False),
        keep_unused=True,
    )
    r = dict(fn=fn, in_names=in_names, out_names=out_names,
             out_avals=out_avals, zero_outs=zero_outs, n_cores=n_cores)
    _RUNNER_CACHE["r"] = r
    return r


def kernel(**inputs):
    r = _get_runner()
    n_cores = r["n_cores"]
    in_maps = [make_in_map(inputs, b % 4) for b in range(n_cores)]
    per_core = [[np.asarray(m[name]) for name in r["in_names"]]
                for m in in_maps]
    concat_in = [
        np.concatenate([per_core[c][i] for c in range(n_cores)], axis=0)
        for i in range(len(r["in_names"]))
    ]
    concat_zeros = [
        np.zeros((n_cores * z.shape[0], *z.shape[1:]), z.dtype)
        for z in r["zero_outs"]
    ]
    out_arrs = r["fn"](*concat_in, *concat_zeros)
    yi = r["out_names"].index("yb")
    full = np.asarray(out_arrs[yi]).reshape(n_cores, L, C)
    out = np.stack([full[b] for b in range(4)])
    return out.reshape(4, 64, 64, C).astype(np.float32)



# revision 20
# speedup vs baseline: 104.0380x; 104.0380x over previous
"""DeSegaMamba (selective-scan vision Mamba) Trainium2 kernel.

kernel(**inputs) takes the FULL inputs of reference.setup_inputs() and
returns the FULL [4, 64, 64, 192] float32 output, running on 8 NeuronCores
(batch-parallel; cores 4-7 mirror batches 0-3).

Layout notes vs v0:
 - x / prompt arrive pre-transposed (host) as bf16 [192, L] split 128+64.
 - output leaves as yT [192, L] f32 (host transposes back).
 - B/C rows bounce through one interleaved DRAM tensor and come back as a
   single [128, 2, TC] partition-broadcast DMA issued on the sync queue
   (HWDGE) instead of gpsimd (SWDGE) to keep the Pool engine free.
 - scan work split: scans mostly on Pool, elementwise mults on DVE, exps on
   Act; LN/gate chain in bf16 for DVE 2x/4x modes.
"""
import numpy as np

"""Patch TileContext._drain_and_barrier: this container's walrus rejects >1
sync-wait per instruction; split the final drain's global-clock waits across
single-wait NOPs on the sync engine."""
import concourse.tile as _tile
from concourse.vector_clock import ScopedClock, VectorClock


def _split_drain_and_barrier(self, tick_clock, wait_clock):
    nc = self.nc
    vclock = tick_clock.global_clock
    n = len(vclock)
    for proc in range(n):
        if vclock[proc] > 0:
            nop = nc.sync.nop(nofuse=True)
            vc = VectorClock([vclock[p] if p == proc else 0 for p in range(n)])
            wait_clock.add_sem_waits(nop.ins, ScopedClock({None: vc}))
    nc.sync.drain()
    nc.all_engine_barrier()
    popped = nc._tile_sem_poison_stack.pop()
    assert popped is self._sem_poison
    nc.clear_and_free_semaphores(list(self.sems.allocated().values()))
    nc.all_engine_barrier()


def apply():
    _tile.TileContext._drain_and_barrier = _split_drain_and_barrier


def legalize_waits(nc, max_waits=1):
    """Walrus (this container's version) rejects >1 sync-wait per instruction.
    Split extras onto preceding same-engine NoOps."""
    from concourse import mybir
    n_split = 0
    for f in nc.m.functions:
        for blk in f.blocks:
            insts = blk.instructions
            out = []
            changed = False
            for inst in insts:
                si = inst.sync_info
                if si is not None and si.on_wait and len(si.on_wait) > max_waits:
                    waits = list(si.on_wait)
                    extra, keep = waits[:-max_waits], waits[-max_waits:]
                    for w_i, w in enumerate(extra):
                        nop = mybir.InstNoOp(
                            name=f"{inst.name}_wsplit{w_i}", ins=[], outs=[])
                        nop.engine = inst.engine
                        nop.sync_info = mybir.SyncInfo(on_wait=[w],
                                                       on_update=[])
                        nc.register_instruction(nop, overwrite=True)
                        out.append(nop)
                    si.on_wait = keep
                    inst.sync_info = si
                    changed = True
                    n_split += 1
                out.append(inst)
            if changed:
                blk.instructions = out
    return n_split


from contextlib import ExitStack

import concourse.bass as bass
import concourse.tile as tile
from concourse import mybir

F32 = mybir.dt.float32
BF16 = mybir.dt.bfloat16
AF = mybir.ActivationFunctionType
OP = mybir.AluOpType

L = 4096
LH = L // 2      # per-core output half
C = 192          # d_model
DI = 384         # d_inner
NS = 16          # d_state
NSL = 8          # states handled per core (pair-split)
RK = 12          # dt_rank
WIMG = 64        # image width (and height)
TC = 512         # time chunk
NCH = L // TC    # 8
NJ = 3           # d_inner row blocks of 128
PAD = 66         # left/right pad of xiT
SCAN_DVE_N = 2   # states whose scan runs on DVE (rest on Pool)
RG = [[0, 1], [2, 3], [4, 5], [6, 7]]  # core pairs (one batch each)


def build_kernel(cfg=None):
    cfg = cfg or {}
    h_dt = BF16
    x_dt = BF16
    bc_dt = BF16

    nc = bass.Bass(num_devices=8)

    # ---- DRAM I/O ----
    xTa = nc.dram_tensor("xTa", [128, L], BF16, kind="ExternalInput")
    xTb = nc.dram_tensor("xTb", [64, L], BF16, kind="ExternalInput")
    pTa = nc.dram_tensor("pTa", [128, L], BF16, kind="ExternalInput")
    pTb = nc.dram_tensor("pTb", [64, L], BF16, kind="ExternalInput")
    W_in = nc.dram_tensor("W_in", [C, 2 * DI], F32, kind="ExternalInput")
    cw9 = nc.dram_tensor("cw9", [DI, 9], F32, kind="ExternalInput")
    wcols = nc.dram_tensor("wcols", [DI, 5], F32, kind="ExternalInput")
    Wx = nc.dram_tensor("Wx", [DI, RK + 2 * NSL], F32, kind="ExternalInput")
    Wdt = nc.dram_tensor("Wdt", [RK, DI], F32, kind="ExternalInput")
    Alog = nc.dram_tensor("Alog", [DI, NSL], F32, kind="ExternalInput")
    Wp = nc.dram_tensor("Wp", [C, NSL], F32, kind="ExternalInput")
    Wout = nc.dram_tensor("Wout", [DI, C], F32, kind="ExternalInput")
    eye = nc.dram_tensor("eye128", [128, 128], F32, kind="ExternalInput")
    cwdiag = nc.dram_tensor("cwdiag", [27, 128, 128], BF16,
                            kind="ExternalInput")
    yTa = nc.dram_tensor("yTa", [128, LH], F32, kind="ExternalOutput")
    yTb = nc.dram_tensor("yTb", [64, LH], F32, kind="ExternalOutput")

    # internal DRAM bounce tensors (partition-broadcast sources)
    bc_d = nc.dram_tensor("bc_d", [NSL, 2, L], bc_dt)
    ms_d = nc.dram_tensor("ms_d", [2, LH], BF16)
    # pair-merge payload: rows 0:384 = y partial (incl. u*D/2),
    # rows 384:768 = z/2.  Index 0 of the leading dim is the pair's first
    # core's time half; ReduceScatter(add) over the pair leaves each core
    # the summed payload for its own half.
    rs_in0 = nc.dram_tensor("rs_in0", [2, 2 * DI, 2 * TC], BF16)
    rs_in1 = nc.dram_tensor("rs_in1", [2, 2 * DI, 2 * TC], BF16)
    rs_out0 = nc.dram_tensor("rs_out0", [2 * DI, 2 * TC], BF16)
    rs_out1 = nc.dram_tensor("rs_out1", [2 * DI, 2 * TC], BF16)

    if cfg.get("trivial"):
        with tile.TileContext(nc) as tc, ExitStack() as ctx:
            pool = ctx.enter_context(tc.tile_pool(name="triv", bufs=4))
            for i in range(NCH // 2):
                t = pool.tile([128, TC], BF16, tag="t", name=f"t{i}")
                nc.sync.dma_start(out=t, in_=xTa[:, i * TC:(i + 1) * TC])
                t2 = pool.tile([128, TC], F32, tag="t2", name=f"t2{i}")
                nc.vector.tensor_copy(out=t2, in_=t)
                nc.sync.dma_start(out=yTa[:, i * TC:(i + 1) * TC], in_=t2)
                u = pool.tile([64, TC], BF16, tag="u", name=f"u{i}")
                nc.sync.dma_start(out=u, in_=xTb[:, i * TC:(i + 1) * TC])
                u2 = pool.tile([64, TC], F32, tag="u2", name=f"u2{i}")
                nc.vector.tensor_copy(out=u2, in_=u)
                nc.sync.dma_start(out=yTb[:, i * TC:(i + 1) * TC], in_=u2)
        return nc

    with tile.TileContext(nc) as tc, ExitStack() as ctx:
        wpool = ctx.enter_context(tc.tile_pool(name="weights", bufs=1))
        upool = ctx.enter_context(tc.tile_pool(name="upool", bufs=1))
        dpool = ctx.enter_context(tc.tile_pool(name="dpool", bufs=1))
        ld = ctx.enter_context(tc.tile_pool(name="ld", bufs=2))
        sm = ctx.enter_context(tc.tile_pool(name="sm", bufs=2))
        statp = ctx.enter_context(tc.tile_pool(name="statp", bufs=4))
        ps_tr = ctx.enter_context(tc.tile_pool(name="ps_tr", bufs=2, space="PSUM"))
        ps_mm = ctx.enter_context(tc.tile_pool(name="ps_mm", bufs=3, space="PSUM"))
        ps_y = ctx.enter_context(tc.tile_pool(name="ps_y", bufs=1, space="PSUM"))

        # ---------------- weights to SBUF ----------------
        eye_t = wpool.tile([128, 128], F32, tag="eye")
        nc.gpsimd.dma_start(out=eye_t, in_=eye[:, :])
        eye_b = wpool.tile([128, 128], BF16, tag="eyeb")
        nc.vector.tensor_copy(out=eye_b, in_=eye_t)
        ones_b = wpool.tile([128, 1], BF16, tag="ones")
        nc.vector.memset(ones_b, 1.0)
        zcol = wpool.tile([128, 1], F32, tag="zcol")
        nc.vector.memset(zcol, 0.0)
        epscol = wpool.tile([1, 1], F32, tag="eps")
        nc.vector.memset(epscol, 1e-5)

        cw_t, cwn_t, An_t, Wx_t, Wout_t = [], [], [], [], []
        # packed loads: [DI, X] -> [128, NJ, X] in one DMA each
        def packed(dram, X, tg, dt=F32):
            t = wpool.tile([128, NJ, X], dt, tag=tg, name=tg)
            ap = dram[0:1, 0:1]
            nc.gpsimd.dma_start(out=t, in_=bass.AP(
                tensor=ap.tensor, offset=0,
                ap=[[X, 128], [128 * X, NJ], [1, X]]))
            return t

        cw_all = packed(cw9, 9, "cw_all")
        wc_all = packed(wcols, 5, "wc_all")
        al_all = packed(Alog, NSL, "al_all")
        wx_allf = packed(Wx, RK + 2 * NSL, "wx_allf")
        wo_allf = packed(Wout, C, "wo_allf")
        wx_all = wpool.tile([128, NJ, RK + 2 * NSL], BF16, tag="wx_all")
        nc.vector.tensor_copy(out=wx_all, in_=wx_allf)
        wo_all = wpool.tile([128, NJ, C], BF16, tag="wo_all")
        nc.vector.tensor_copy(out=wo_all, in_=wo_allf)
        cwd_all = wpool.tile([128, 27, 128], BF16, tag="cwd_all")
        ap = cwdiag[0:1, 0:1, 0:1]
        nc.gpsimd.dma_start(out=cwd_all, in_=bass.AP(
            tensor=ap.tensor, offset=0,
            ap=[[128, 128], [16384, 27], [1, 128]]))
        cwd_t = [[cwd_all[:, j * 9 + tap, :] for tap in range(9)]
                 for j in range(NJ)]
        an_all = wpool.tile([128, NJ, NSL], F32, tag="an_all")
        nc.scalar.activation(out=an_all, in_=al_all, func=AF.Exp)
        nc.vector.tensor_scalar_mul(out=an_all, in0=an_all, scalar1=-1.0)
        cwn_all = wpool.tile([128, NJ, 9], F32, tag="cwn_all")
        nc.vector.tensor_scalar_mul(out=cwn_all, in0=cw_all, scalar1=-1.0)
        cb_t, bdt_t, D_t, g_t, b_t = [], [], [], [], []
        for j in range(NJ):
            cw_t.append(cw_all[:, j, :])
            cwn_t.append(cwn_all[:, j, :])
            cb_t.append(wc_all[:, j, 0:1])
            bdt_t.append(wc_all[:, j, 1:2])
            D_t.append(wc_all[:, j, 2:3])
            g_t.append(wc_all[:, j, 3:4])
            b_t.append(wc_all[:, j, 4:5])
            An_t.append(an_all[:, j, :])
            Wx_t.append(wx_all[:, j, :])
            Wout_t.append(wo_all[:, j, :])
        Wdt_t = wpool.tile([RK, DI], F32, tag="wdt")
        nc.gpsimd.dma_start(out=Wdt_t, in_=Wdt[:, :])
        Wp_af = wpool.tile([128, NSL], F32, tag="wp_af")
        nc.gpsimd.dma_start(out=Wp_af, in_=Wp[0:128, :])
        Wp_bf = wpool.tile([64, NSL], F32, tag="wp_bf")
        nc.gpsimd.dma_start(out=Wp_bf, in_=Wp[128:192, :])
        Wp_a = wpool.tile([128, NSL], BF16, tag="wp_a")
        nc.vector.tensor_copy(out=Wp_a, in_=Wp_af)
        Wp_b = wpool.tile([64, NSL], BF16, tag="wp_b")
        nc.vector.tensor_copy(out=Wp_b, in_=Wp_bf)

        uT16 = [upool.tile([128, L], BF16, tag=f"uT16{j}", name=f"uT16{j}")
                for j in range(NJ)]
        deltaT = [[dpool.tile([128, TC], BF16, tag=f"deltaT{j}_{k}",
                              name=f"deltaT{j}_{k}") for k in range(NCH)]
                  for j in range(NJ)]

        PW = PAD + L + PAD

        with tc.tile_pool(name="xipool", bufs=1) as xipool:
            xiT = [xipool.tile([128, PW], BF16, tag=f"xiT{j}",
                               name=f"xiT{j}") for j in range(NJ)]

            with tc.tile_pool(name="xtpool", bufs=1) as xtpool:
                xTa_t = xtpool.tile([128, L], BF16, tag="xTa_t")
                nc.sync.dma_start(out=xTa_t, in_=xTa[:, :])
                xTb_t = xtpool.tile([64, L], BF16, tag="xTb_t")
                nc.sync.dma_start(out=xTb_t, in_=xTb[:, :])
                Wi_af = xtpool.tile([128, 2 * DI], F32, tag="Wi_af")
                Wi_bf = xtpool.tile([64, 2 * DI], F32, tag="Wi_bf")
                nc.gpsimd.dma_start(out=Wi_af, in_=W_in[0:128, :])
                nc.gpsimd.dma_start(out=Wi_bf, in_=W_in[128:192, :])
                Wi_a = xtpool.tile([128, 2 * DI], BF16, tag="Wi_a")
                Wi_b = xtpool.tile([64, 2 * DI], BF16, tag="Wi_b")
                nc.vector.tensor_copy(out=Wi_a, in_=Wi_af)
                nc.vector.tensor_copy(out=Wi_b, in_=Wi_bf)

                # ---- phase 2: xz projection ----
                for j in range(NJ):
                    nc.vector.memset(xiT[j][:, 0:PAD], 0.0)
                    nc.vector.memset(xiT[j][:, PAD + L:PW], 0.0)
                for j in range(NJ):
                    c0 = j * 128
                    for k in range(NCH):
                        t0 = k * TC
                        pxi = ps_mm.tile([128, TC], F32, tag="pmm")
                        nc.tensor.matmul(pxi,
                                         Wi_a[:, c0:c0 + 128],
                                         xTa_t[:, t0:t0 + TC],
                                         start=True, stop=False)
                        nc.tensor.matmul(pxi,
                                         Wi_b[:, c0:c0 + 128],
                                         xTb_t[:, t0:t0 + TC],
                                         start=False, stop=True)
                        nc.scalar.copy(
                            out=xiT[j][:, PAD + t0:PAD + t0 + TC],
                            in_=pxi)
                        pz = ps_mm.tile([128, TC], F32, tag="pmm")
                        nc.tensor.matmul(
                            pz, Wi_a[:, DI + c0:DI + c0 + 128],
                            xTa_t[:, t0:t0 + TC],
                            start=True, stop=False)
                        nc.tensor.matmul(
                            pz, Wi_b[:, DI + c0:DI + c0 + 128],
                            xTb_t[:, t0:t0 + TC],
                            start=False, stop=True)
                        zs = sm.tile([128, TC], BF16, tag="zs",
                                      name=f"zs{j}_{k}")
                        nc.scalar.activation(out=zs, in_=pz, func=AF.Silu)
                        zh = sm.tile([128, TC], BF16, tag="zh",
                                     name=f"zh{j}_{k}")
                        nc.vector.tensor_scalar_mul(out=zh, in0=zs,
                                                    scalar1=0.5)
                        rs_t = rs_in0 if (k % 4) // 2 == 0 else rs_in1
                        hh, cc0 = k // 4, (k % 2) * TC
                        nc.sync.dma_start(
                            out=rs_t[hh:hh + 1,
                                     DI + j * 128:DI + (j + 1) * 128,
                                     cc0:cc0 + TC],
                            in_=zh)

            # ---- phase 3: depthwise conv via diagonal matmuls on PE ----
            for j in range(NJ):
                for k in range(NCH):
                    t0 = k * TC
                    pc = ps_mm.tile([128, TC], F32, tag="pmm",
                                    name=f"pconv{j}_{k}")
                    for ky in range(3):
                        for kx in range(3):
                            dy, dx = ky - 1, kx - 1
                            tap = ky * 3 + kx
                            off = PAD + t0 + dy * WIMG + dx
                            nc.tensor.matmul(
                                pc, cwd_t[j][tap],
                                xiT[j][:, off:off + TC],
                                start=(tap == 0), stop=(tap == 8))
                    acc = ld.tile([128, TC], F32, tag="cacc",
                                  name=f"cacc{j}_{k}")
                    nc.scalar.copy(out=acc, in_=pc)
                    # wrap fixes within this chunk (8 image rows):
                    # dx=+1 contaminates w=63, dx=-1 contaminates w=0
                    a3 = acc.rearrange("p (h w) -> p h w", w=WIMG)
                    xi3 = xiT[j]
                    r0 = t0 // WIMG  # first image row of this chunk
                    eng = nc.vector
                    for ky in range(3):
                        dy = ky - 1

                        def stv(start):
                            base = xi3[:, start:start + 1]
                            return bass.AP(
                                tensor=base.tensor, offset=base.offset,
                                ap=[list(base.ap[0]),
                                    [WIMG, TC // WIMG]])

                        tap = ky * 3 + 2   # dx=+1 read xi[h+dy+1, 0]
                        eng.scalar_tensor_tensor(
                            out=a3[:, :, WIMG - 1],
                            in0=stv(PAD + (r0 + dy + 1) * WIMG),
                            scalar=cwn_t[j][:, tap:tap + 1],
                            in1=a3[:, :, WIMG - 1],
                            op0=OP.mult, op1=OP.add)
                        tap = ky * 3       # dx=-1 read xi[h+dy-1, 63]
                        eng.scalar_tensor_tensor(
                            out=a3[:, :, 0],
                            in0=stv(PAD + (r0 + dy) * WIMG - 1),
                            scalar=cwn_t[j][:, tap:tap + 1],
                            in1=a3[:, :, 0],
                            op0=OP.mult, op1=OP.add)
                    nc.scalar.activation(out=uT16[j][:, t0:t0 + TC],
                                         in_=acc, func=AF.Silu,
                                         bias=cb_t[j])

        # ---- phase 4: x_dbl, delta, B, C ----
        with tc.tile_pool(name="ptpool", bufs=1) as ptpool:
            pT_a = ptpool.tile([128, L], BF16, tag="pT_a")
            nc.sync.dma_start(out=pT_a, in_=pTa[:, :])
            pT_b = ptpool.tile([64, L], BF16, tag="pT_b")
            nc.sync.dma_start(out=pT_b, in_=pTb[:, :])
            dtrT = ptpool.tile([RK, L], F32, tag="dtrT")

            for k in range(NCH):
                t0 = k * TC
                p_dtr = ps_mm.tile([RK, TC], F32, tag="pmm")
                p_B = ps_mm.tile([NSL, TC], F32, tag="pmm")
                p_C = ps_mm.tile([NSL, TC], F32, tag="pmm")
                for j in range(NJ):
                    nc.tensor.matmul(p_dtr, Wx_t[j][:, 0:RK],
                                     uT16[j][:, t0:t0 + TC],
                                     start=(j == 0), stop=(j == NJ - 1))
                for j in range(NJ):
                    nc.tensor.matmul(p_B, Wx_t[j][:, RK:RK + NSL],
                                     uT16[j][:, t0:t0 + TC],
                                     start=(j == 0), stop=(j == NJ - 1))
                for j in range(NJ):
                    nc.tensor.matmul(
                        p_C, Wx_t[j][:, RK + NSL:RK + 2 * NSL],
                        uT16[j][:, t0:t0 + TC],
                        start=(j == 0), stop=False)
                nc.tensor.matmul(p_C, Wp_a[:, :],
                                 pT_a[:, t0:t0 + TC],
                                 start=False, stop=False)
                nc.tensor.matmul(p_C, Wp_b[:, :],
                                 pT_b[:, t0:t0 + TC],
                                 start=False, stop=True)
                nc.scalar.copy(out=dtrT[:, t0:t0 + TC], in_=p_dtr)
                bt = sm.tile([NSL, TC], bc_dt, tag="bs16")
                nc.scalar.copy(out=bt, in_=p_B)
                nc.sync.dma_start(out=bass.AP(
                    tensor=bc_d[0:1, 0:1, 0:1].tensor, offset=t0,
                    ap=[[2 * L, NSL], [1, TC]]), in_=bt)
                ct = sm.tile([NSL, TC], bc_dt, tag="bs16")
                nc.scalar.copy(out=ct, in_=p_C)
                nc.sync.dma_start(out=bass.AP(
                    tensor=bc_d[0:1, 0:1, 0:1].tensor, offset=L + t0,
                    ap=[[2 * L, NSL], [1, TC]]), in_=ct)

            for j in range(NJ):
                for k in range(NCH):
                    t0 = k * TC
                    pd = ps_mm.tile([128, TC], F32, tag="pmm")
                    nc.tensor.matmul(
                        pd, Wdt_t[:, j * 128:(j + 1) * 128],
                        dtrT[:, t0:t0 + TC],
                        start=True, stop=True)
                    # softplus(x) = ln(1 + exp(x)) via Exp then Ln
                    esb = sm.tile([128, TC], F32, tag="esb",
                                  name=f"esb{j}_{k}")
                    nc.scalar.activation(out=esb, in_=pd, func=AF.Exp,
                                         bias=bdt_t[j])
                    nc.scalar.activation(out=deltaT[j][k],
                                         in_=esb, func=AF.Ln, bias=1.0)

        # ---------------- phase 6+7: scan + LN + gate + out ----------------
        hpool = ctx.enter_context(tc.tile_pool(name="hpool", bufs=1))
        sc = ctx.enter_context(tc.tile_pool(name="scan", bufs=6))
        scx = ctx.enter_context(tc.tile_pool(name="scanx", bufs=4))
        scW = ctx.enter_context(tc.tile_pool(name="scanw", bufs=2))
        bc = ctx.enter_context(tc.tile_pool(name="bc", bufs=3))
        bcmu = ctx.enter_context(tc.tile_pool(name="bcmu", bufs=1))
        yc_pool = ctx.enter_context(tc.tile_pool(name="ycp", bufs=3))
        ysq_pool = ctx.enter_context(tc.tile_pool(name="ysqp", bufs=2))
        zl_pool = ctx.enter_context(tc.tile_pool(name="zl", bufs=2))
        out_pool = ctx.enter_context(tc.tile_pool(name="outp", bufs=2))
        hst = [[hpool.tile([128, TC], h_dt, tag=f"h{j}_{n}",
                           name=f"h{j}_{n}") for n in range(NSL)]
               for j in range(NJ)]
        for k in range(NCH):
            t0 = k * TC
            py = [ps_y.tile([128, TC], F32, tag=f"py{j}", name=f"py{j}_{k}")
                  for j in range(NJ)]
            wch = []
            for j in range(NJ):
                w_t = scW.tile([128, TC], BF16, tag=f"w{j}", name=f"w{j}_{k}")
                nc.gpsimd.tensor_tensor(out=w_t, in0=deltaT[j][k],
                                        in1=uT16[j][:, t0:t0 + TC],
                                        op=OP.mult)
                wch.append(w_t)
            for n in range(NSL):
                bct = bc.tile([128, 2, TC], bc_dt, tag="bct")
                bsrc = bc_d[n:n + 1, 0:1, t0:t0 + TC]
                nc.sync.dma_start(out=bct, in_=bass.AP(
                    tensor=bsrc.tensor, offset=bsrc.offset,
                    ap=[[0, 128], [L, 2], [1, TC]]))
                brow = bct[:, 0, :]
                crow = bct[:, 1, :]
                for j in range(NJ):
                    a_t = sc.tile([128, TC], F32, tag="a")
                    nc.scalar.activation(out=a_t, in_=deltaT[j][k],
                                         func=AF.Exp,
                                         scale=An_t[j][:, n:n + 1])
                    x_t = scx.tile([128, TC], x_dt, tag="x")
                    xe = nc.gpsimd if j == 2 else nc.vector
                    xe.tensor_tensor(out=x_t, in0=wch[j], in1=brow,
                                     op=OP.mult)
                    h = hst[j][n]
                    init = zcol[:, 0:1] if k == 0 else h[:, TC - 1:TC]
                    nc.vector.tensor_tensor_scan(
                        out=h, data0=a_t, data1=x_t, initial=init,
                        op0=OP.mult, op1=OP.add)
                    ym = scx.tile([128, TC], h_dt, tag="ym")
                    ye = nc.gpsimd if j == 2 else nc.vector
                    ye.tensor_tensor(out=ym, in0=h, in1=crow,
                                     op=OP.mult)
                    nc.tensor.matmul(py[j], eye_b, ym,
                                     start=(n == 0), stop=(n == NSL - 1))

            # y partial chunk = py + u*(D/2) -> pair-merge payload
            rs_t = rs_in0 if (k % 4) // 2 == 0 else rs_in1
            hh, cc0 = k // 4, (k % 2) * TC
            for j in range(NJ):
                t = yc_pool.tile([128, TC], BF16, tag="yc", name=f"yc{j}_{k}")
                nc.vector.scalar_tensor_tensor(
                    out=t, in0=uT16[j][:, t0:t0 + TC],
                    scalar=D_t[j], in1=py[j],
                    op0=OP.mult, op1=OP.add)
                nc.sync.dma_start(
                    out=rs_t[hh:hh + 1, j * 128:(j + 1) * 128, cc0:cc0 + TC],
                    in_=t)
            # pair merge: each core receives its own half's summed y+z.
            # Issue each ReduceScatter as soon as its quarter-chunks are
            # written so it overlaps the remaining scan chunks.
            if k == 5:
                nc.gpsimd.collective_compute(
                    "ReduceScatter", OP.add, replica_groups=RG,
                    ins=[rs_in0[:, :, :].opt()], outs=[rs_out0[:, :].opt()])
            if k == 7:
                nc.gpsimd.collective_compute(
                    "ReduceScatter", OP.add, replica_groups=RG,
                    ins=[rs_in1[:, :, :].opt()], outs=[rs_out1[:, :].opt()])

        # ---------------- finalize my half: LN + gate + out ----------------
        for k2 in range(NCH // 2):
            rs_o = rs_out0 if k2 < 2 else rs_out1
            cc = (k2 % 2) * TC
            t0 = k2 * TC  # position within my half
            yl, zl, ysq = [], [], []
            p_s = ps_mm.tile([1, TC], F32, tag="pmm")
            p_q = ps_mm.tile([1, TC], F32, tag="pmm")
            for j in range(NJ):
                yld = yc_pool.tile([128, TC], BF16, tag="yl",
                                   name=f"yl{j}_{k2}")
                nc.sync.dma_start(
                    out=yld, in_=rs_o[j * 128:(j + 1) * 128, cc:cc + TC])
                yl.append(yld)
                zld = zl_pool.tile([128, TC], BF16, tag="zl",
                                   name=f"zl{j}_{k2}")
                nc.sync.dma_start(
                    out=zld,
                    in_=rs_o[DI + j * 128:DI + (j + 1) * 128, cc:cc + TC])
                zl.append(zld)
                q = ysq_pool.tile([128, TC], BF16, tag="ysq",
                                  name=f"ysq{j}_{k2}")
                nc.gpsimd.tensor_tensor(out=q, in0=yld, in1=yld, op=OP.mult)
                ysq.append(q)
            for j in range(NJ):
                nc.tensor.matmul(p_s, ones_b[:, 0:1], yl[j],
                                 start=(j == 0), stop=(j == NJ - 1))
            for j in range(NJ):
                nc.tensor.matmul(p_q, ones_b[:, 0:1], ysq[j],
                                 start=(j == 0), stop=(j == NJ - 1))
            mu = statp.tile([1, TC], F32, tag="stat", name=f"mu{k2}")
            nc.scalar.mul(out=mu, in_=p_s, mul=1.0 / DI)
            msq = statp.tile([1, TC], F32, tag="stat", name=f"msq{k2}")
            nc.scalar.mul(out=msq, in_=p_q, mul=1.0 / DI)
            var = statp.tile([1, TC], F32, tag="stat", name=f"var{k2}")
            nc.vector.tensor_tensor(out=var, in0=mu, in1=mu, op=OP.mult)
            nc.vector.tensor_tensor(out=var, in0=msq, in1=var, op=OP.subtract)
            lnv = statp.tile([1, TC], F32, tag="stat", name=f"lnv{k2}")
            nc.scalar.activation(out=lnv, in_=var, func=AF.Ln,
                                 bias=epscol[0:1, 0:1])
            mu16 = statp.tile([1, TC], BF16, tag="stat", name=f"mu16{k2}")
            nc.vector.tensor_copy(out=mu16, in_=mu)
            rstd = statp.tile([1, TC], BF16, tag="stat", name=f"rstd{k2}")
            nc.scalar.activation(out=rstd, in_=lnv, func=AF.Exp, scale=-0.5)
            nc.sync.dma_start(out=ms_d[0:1, t0:t0 + TC], in_=mu16)
            nc.sync.dma_start(out=ms_d[1:2, t0:t0 + TC], in_=rstd)
            msb = bcmu.tile([128, 2, TC], BF16, tag="msb")
            src = ms_d[0:1, t0:t0 + TC]
            nc.sync.dma_start(out=msb, in_=bass.AP(
                tensor=src.tensor, offset=src.offset,
                ap=[[0, 128], [LH, 2], [1, TC]]))
            mu_b = msb[:, 0, :]
            rs_b = msb[:, 1, :]

            po_a = ps_mm.tile([128, TC], F32, tag="pmm")
            po_b = ps_mm.tile([64, TC], F32, tag="pmm")
            for j in range(NJ):
                g1 = out_pool.tile([128, TC], BF16, tag="g1",
                                   name=f"g1{j}_{k2}")
                g16 = out_pool.tile([128, TC], BF16, tag="g16",
                                    name=f"g16{j}_{k2}")
                ln_eng = nc.gpsimd if j >= 1 else nc.vector
                ln_eng.tensor_tensor(out=g1, in0=yl[j], in1=mu_b,
                                     op=OP.subtract)
                ln_eng.tensor_tensor(out=g1, in0=g1, in1=rs_b, op=OP.mult)
                nc.vector.tensor_scalar(out=g1, in0=g1,
                                        scalar1=g_t[j],
                                        scalar2=b_t[j],
                                        op0=OP.mult, op1=OP.add)
                ln_eng.tensor_tensor(out=g16, in0=g1, in1=zl[j], op=OP.mult)
                nc.tensor.matmul(po_a, Wout_t[j][:, 0:128], g16,
                                 start=(j == 0), stop=(j == NJ - 1))
                nc.tensor.matmul(po_b, Wout_t[j][:, 128:192], g16,
                                 start=(j == 0), stop=(j == NJ - 1))
            so_a = out_pool.tile([128, TC], F32, tag="so", name=f"so_a{k2}")
            nc.scalar.copy(out=so_a, in_=po_a)
            so_b = out_pool.tile([64, TC], F32, tag="so", name=f"so_b{k2}")
            nc.scalar.copy(out=so_b, in_=po_b)
            nc.sync.dma_start(out=yTa[:, t0:t0 + TC], in_=so_a)
            nc.sync.dma_start(out=yTb[:, t0:t0 + TC], in_=so_b)

    return nc


def make_in_map(inputs, b, half):
    """Per-core input dict: batch b, state-half `half` (0: n 0-7, 1: n 8-15)."""
    import ml_dtypes
    f32 = np.float32
    bf16 = ml_dtypes.bfloat16
    n0 = half * NSL
    x = np.asarray(inputs["x"], f32)
    prompt = np.asarray(inputs["prompt"], f32)
    xT = np.ascontiguousarray(x[b].reshape(L, C).T.astype(bf16))
    pT = np.ascontiguousarray(prompt[b].reshape(L, C).T.astype(bf16))
    wcols = np.stack([
        np.asarray(inputs["conv_b"], f32).reshape(DI),
        np.asarray(inputs["b_dt"], f32).reshape(DI),
        np.asarray(inputs["D"], f32).reshape(DI) * 0.5,
        np.asarray(inputs["ln_g"], f32).reshape(DI),
        np.asarray(inputs["ln_b"], f32).reshape(DI),
    ], axis=1)  # [DI, 5]
    Wx = np.asarray(inputs["Wx"], f32)
    Wx_loc = np.concatenate([
        Wx[:, 0:RK],
        Wx[:, RK + n0:RK + n0 + NSL],
        Wx[:, RK + NS + n0:RK + NS + n0 + NSL],
    ], axis=1)  # [DI, RK + 2*NSL]
    return {
        "xTa": np.ascontiguousarray(xT[0:128]),
        "xTb": np.ascontiguousarray(xT[128:192]),
        "pTa": np.ascontiguousarray(pT[0:128]),
        "pTb": np.ascontiguousarray(pT[128:192]),
        "W_in": np.asarray(inputs["W_in"], f32),
        "cw9": np.ascontiguousarray(np.asarray(inputs["conv_w"], f32)
                                    .reshape(DI, 9)),
        "wcols": np.ascontiguousarray(wcols),
        "Wx": np.ascontiguousarray(Wx_loc),
        "Wdt": np.asarray(inputs["Wdt"], f32),
        "Alog": np.ascontiguousarray(
            np.asarray(inputs["A_log"], f32)[:, n0:n0 + NSL]),
        "Wp": np.ascontiguousarray(
            np.asarray(inputs["Wp"], f32)[:, n0:n0 + NSL]),
        "Wout": np.asarray(inputs["Wout"], f32),
        "eye128": np.eye(128, dtype=f32),
        "cwdiag": _cwdiag(np.asarray(inputs["conv_w"], f32).reshape(DI, 9)),
    }


def _cwdiag(cw9):
    import ml_dtypes
    out = np.zeros((27, 128, 128), dtype=ml_dtypes.bfloat16)
    for j in range(NJ):
        for tap in range(9):
            np.fill_diagonal(out[j * 9 + tap],
                             cw9[j * 128:(j + 1) * 128, tap]
                             .astype(ml_dtypes.bfloat16))
    return out


_RUNNER_CACHE = {}


def _get_runner():
    if "r" in _RUNNER_CACHE:
        return _RUNNER_CACHE["r"]
    apply()  # tile drain patch
    import jax
    from jax.experimental.shard_map import shard_map
    from jax.sharding import Mesh, PartitionSpec
    from concourse import bass2jax as b2j

    nc = build_kernel()
    legalize_waits(nc)
    b2j.install_neuronx_cc_hook()
    partition_name = (nc.partition_id_tensor.name
                      if nc.partition_id_tensor else None)
    in_names, out_names, out_avals, zero_outs = [], [], [], []
    for alloc in nc.m.functions[0].allocations:
        if not isinstance(alloc, mybir.MemoryLocationSet):
            continue
        name = alloc.memorylocations[0].name
        if alloc.kind == "ExternalInput":
            if name != partition_name:
                in_names.append(name)
        elif alloc.kind == "ExternalOutput":
            out_names.append(name)
            shape = tuple(alloc.tensor_shape)
            dtype = mybir.dt.np(alloc.dtype)
            out_avals.append(jax.core.ShapedArray(shape, dtype))
            zero_outs.append(np.zeros(shape, dtype))
    n_params = len(in_names)
    all_in_names = list(in_names) + list(out_names)
    if partition_name is not None:
        all_in_names.append(partition_name)

    def _body(*args):
        operands = list(args)
        if partition_name is not None:
            operands.append(b2j.partition_id_tensor())
        outs = b2j._bass_exec_p.bind(
            *operands,
            out_avals=tuple(out_avals),
            in_names=tuple(all_in_names),
            out_names=tuple(out_names),
            lowering_input_output_aliases=(),
            sim_require_finite=True,
            sim_require_nnan=True,
            nc=nc,
        )
        return tuple(outs)

    n_cores = 8
    devices = jax.devices()[:n_cores]
    mesh = Mesh(np.asarray(devices), ("core",))
    in_specs = (PartitionSpec("core"),) * (n_params + len(out_names))
    out_specs = (PartitionSpec("core"),) * len(out_names)
    fn = jax.jit(
        shard_map(_body, mesh=mesh, in_specs=in_specs,
                  out_specs=out_specs, check_rep=False),
        keep_unused=True,
    )
    r = dict(fn=fn, in_names=in_names, out_names=out_names,
             out_avals=out_avals, zero_outs=zero_outs, n_cores=n_cores)
    _RUNNER_CACHE["r"] = r
    return r


def kernel(**inputs):
    r = _get_runner()
    n_cores = r["n_cores"]
    in_maps = [make_in_map(inputs, c // 2, c % 2) for c in range(n_cores)]
    per_core = [[np.asarray(m[name]) for name in r["in_names"]]
                for m in in_maps]
    concat_in = [
        np.concatenate([per_core[c][i] for c in range(n_cores)], axis=0)
        for i in range(len(r["in_names"]))
    ]
    concat_zeros = [
        np.zeros((n_cores * z.shape[0], *z.shape[1:]), z.dtype)
        for z in r["zero_outs"]
    ]
    out_arrs = r["fn"](*concat_in, *concat_zeros)
    ya = np.asarray(out_arrs[r["out_names"].index("yTa")]).reshape(
        n_cores, 128, LH)
    yb_ = np.asarray(out_arrs[r["out_names"].index("yTb")]).reshape(
        n_cores, 64, LH)
    outs = []
    for b in range(4):
        c0, c1 = 2 * b, 2 * b + 1
        yT = np.concatenate([
            np.concatenate([ya[c0], yb_[c0]], axis=0),
            np.concatenate([ya[c1], yb_[c1]], axis=0),
        ], axis=1)  # [C, L]
        outs.append(yT.T)
    return np.stack(outs).reshape(4, 64, 64, C).astype(np.float32)
